# revision 1
# baseline (speedup 1.0000x reference)
"""TRN2 Bass kernel for nn_Attention_11252814315826.

out[b,h,s,:] = softmax(Q[b,h] @ K^T[b,h] / 8 + addr(mask)) @ V[b,h]
with the additive mask on the QUERY dim: for mask[b,s]==0 the reference's
-1e12 row offset makes softmax exactly uniform, so out = colmean(V[b,h]).

Strategy: shard the 32 (b,h) pairs 4-per-core across 8 NeuronCores
(cores 0-3: b=0, cores 4-7: b=1). Host-side, compact the query rows to
the mask==1 subset (shared per b), zero-padded to SP (multiple of 128).
Zero-padded rows yield exactly uniform attention on device (scores are
exactly 0 -> exp 1), so one padded row's output supplies colmean(V) for
all masked rows; no separate device pass is needed.
"""

import os
import sys

for _p in (
    "/root/.axon_site",
    "/root/.axon_site/_ro/trn_rl_repo",
    "/root/.axon_site/_ro/pypackages",
    "/opt/trn_rl_repo",
):
    if os.path.isdir(_p) and _p not in sys.path:
        sys.path.append(_p)

from concourse.bass_utils import run_bass_kernel_spmd

import numpy as np

import concourse.bacc as bacc
import concourse.tile as tile
import concourse.mybir as mybir

F32 = mybir.dt.float32
F32R = mybir.dt.float32r
BF16 = mybir.dt.bfloat16


def _chunks(total, size):
    out, s0 = [], 0
    while s0 < total:
        w = min(size, total - s0)
        out.append((s0, w))
        s0 += w
    return out


def build_attention_nc(NP=4, SP=1024, S=2048, D=64, CHUNK=512, row_tile=False,
                       repeat=1):
    assert SP % 128 == 0 and S % 256 == 0 and D == 64
    NT = S // 128
    NS = SP // 128

    nc = bacc.Bacc("TRN2", target_bir_lowering=False, debug=False)

    q = nc.dram_tensor("q", [NP, SP, D], F32, kind="ExternalInput")
    kt = nc.dram_tensor("kt", [NP, D, S], BF16, kind="ExternalInput")
    v = nc.dram_tensor("v", [NP, S, D], BF16, kind="ExternalInput")
    o = nc.dram_tensor("o", [NP, SP, D], F32, kind="ExternalOutput")

    ident_dram = nc.inline_tensor(np.eye(128, dtype=np.float32), name="ident")
    import ml_dtypes
    ones_dram = nc.inline_tensor(
        np.ones((128, S // 128, 1), dtype=ml_dtypes.bfloat16), name="onescol"
    )
    dma = nc.sync

    ctxs = {}  # pair -> dict of live tiles

    with tile.TileContext(nc) as tc:
        with (
            tc.tile_pool(name="const", bufs=1) as const_pool,
            tc.tile_pool(name="kt", bufs=2) as kt_pool,
            tc.tile_pool(name="v", bufs=2) as v_pool,
            tc.tile_pool(name="qin", bufs=2) as q_pool,
            tc.tile_pool(name="qt", bufs=2) as qt_pool,
            tc.tile_pool(name="exp", bufs=4) as exp_pool,
            tc.tile_pool(name="osb", bufs=2) as osb_pool,
            tc.tile_pool(name="oout", bufs=2) as oout_pool,
            tc.tile_pool(name="recip", bufs=4) as recip_pool,
            tc.tile_pool(name="qkps", bufs=2, space="PSUM") as qk_psum,
            tc.tile_pool(name="pvps", bufs=2, space="PSUM") as pv_psum,
            tc.tile_pool(name="trps", bufs=2, space="PSUM") as tr_psum,
        ):
            ident = const_pool.tile([128, 128], F32)
            dma.dma_start(ident[:], ident_dram.ap())

            def pair_prologue(p):
                KP = 128 if row_tile else D
                kt_sb = kt_pool.tile([KP, S], BF16)
                for c0 in range(0, S, S // 4):
                    dma.dma_start(
                        kt_sb[0:D, c0 : c0 + S // 4], kt.ap()[p][:, c0 : c0 + S // 4]
                    )
                    if row_tile:
                        dma.dma_start(
                            kt_sb[D : 2 * D, c0 : c0 + S // 4],
                            kt.ap()[p][:, c0 : c0 + S // 4],
                        )

                v_sb = v_pool.tile([128, NT, D + 1], BF16)
                v_src = v.ap()[p].rearrange("(t p) d -> p t d", p=128)
                for t0 in range(0, NT, NT // 4):
                    dma.dma_start(
                        v_sb[:, t0 : t0 + NT // 4, 0:D],
                        v_src[:, t0 : t0 + NT // 4, :],
                    )
                dma.dma_start(v_sb[:, :, D : D + 1], ones_dram.ap())

                q_sb = q_pool.tile([128, NS, D], F32)
                q_src = q.ap()[p].rearrange("(n p) d -> p n d", p=128)
                for n0 in range(0, NS, max(1, NS // 2)):
                    nn_ = min(max(1, NS // 2), NS - n0)
                    dma.dma_start(q_sb[:, n0 : n0 + nn_, :], q_src[:, n0 : n0 + nn_, :])

                qt_sb = qt_pool.tile([KP, SP], BF16)
                for n in range(NS):
                    blk = qt_sb[:, n * 128 : (n + 1) * 128]
                    if row_tile:
                        q_tr = tr_psum.tile([128, 128], F32, tag="trp")
                        nc.tensor.transpose(q_tr[0:D, :], q_sb[:, n, :], ident[:])
                        nc.tensor.transpose(
                            q_tr[D : 2 * D, :],
                            q_sb[:, n, :],
                            ident[:],
                            tile_position=(0, D),
                        )
                        nc.vector.tensor_copy(blk, q_tr[:])
                    else:
                        q_tr = tr_psum.tile([D, 128], F32, tag="trp")
                        nc.tensor.transpose(q_tr[:], q_sb[:, n, :], ident[:])
                        nc.vector.tensor_copy(blk, q_tr[:])
                ctxs[p] = dict(kt=kt_sb, v=v_sb, qt=qt_sb)

            def emit_qk(p, s0, sw, tg):
                """Matmuls for t-blocks 2tg, 2tg+1 -> returns psum tile."""
                cx = ctxs[p]
                kt_sb, qt_sb = cx["kt"], cx["qt"]
                qk_ps = qk_psum.tile([128, 2 * CHUNK], F32, tag="qkp")
                for half in range(2):
                    t = 2 * tg + half
                    off = half * sw
                    if row_tile:
                        mv_a = qt_sb[0:D, s0 : s0 + sw]
                        mv_b = qt_sb[D : 2 * D, s0 : s0 + sw]
                        c0 = t * 128
                        nc.tensor.matmul(
                            qk_ps[0:D, off : off + sw],
                            kt_sb[0:D, c0 : c0 + D],
                            mv_a,
                            start=True,
                            stop=True,
                        )
                        nc.tensor.matmul(
                            qk_ps[D : 2 * D, off : off + sw],
                            kt_sb[D : 2 * D, c0 + D : c0 + 128],
                            mv_b,
                            start=True,
                            stop=True,
                        )
                    else:
                        nc.tensor.matmul(
                            qk_ps[:, off : off + sw],
                            kt_sb[0:D, t * 128 : (t + 1) * 128],
                            qt_sb[0:D, s0 : s0 + sw],
                            start=True,
                            stop=True,
                        )
                return qk_ps

            def emit_exp(p, sw, qk_ps):
                exp_sb = exp_pool.tile([128, 2 * CHUNK], BF16, tag="exp")
                nc.scalar.activation(
                    exp_sb[:, 0 : 2 * sw],
                    qk_ps[:, 0 : 2 * sw],
                    mybir.ActivationFunctionType.Exp,
                    scale=0.125,
                )
                return exp_sb

            def make_pv(p, sw, tg, exp_sb, pv_ps):
                def emit():
                    v_sb = ctxs[p]["v"]
                    for half in range(2):
                        t = 2 * tg + half
                        nc.tensor.matmul(
                            pv_ps[:, 0:sw],
                            v_sb[:, t, :],
                            exp_sb[:, half * sw : (half + 1) * sw],
                            start=(t == 0),
                            stop=(t == NT - 1),
                            skip_group_check=True,
                        )

                return emit

            def make_epilogue(p, s0, sw, pv_ps):
                def emit():
                    o_sb = osb_pool.tile([D + 1, CHUNK], F32, tag="osb")
                    nc.vector.tensor_copy(o_sb[:, 0:sw], pv_ps[:, 0:sw])
                    nsub = sw // 128
                    oout = oout_pool.tile([128, CHUNK // 128, D], F32, tag="oout")
                    for j in range(nsub):
                        o_tr = tr_psum.tile([128, D + 1], F32, tag="trp")
                        nc.tensor.transpose(
                            o_tr[:],
                            o_sb[:, j * 128 : (j + 1) * 128],
                            ident[0 : D + 1, 0 : D + 1],
                        )
                        recip = recip_pool.tile([128, 1], F32, tag="rcp")
                        nc.vector.reciprocal(recip[:], o_tr[:, D : D + 1])
                        nc.vector.tensor_scalar_mul(
                            oout[:, j, :], o_tr[:, 0:D], recip[:]
                        )
                    dma.dma_start(
                        o.ap()[p].rearrange("(n p) d -> p n d", p=128)[
                            :, s0 // 128 : s0 // 128 + nsub, :
                        ],
                        oout[:, 0:nsub, :],
                    )

                return emit

            # ---- flat software-pipelined emission --------------------------
            # Depth-2 pipeline: exp(g) is emitted right after QK(g) (ACT can
            # start as soon as the matmuls land), but PV(g) enters the PE
            # stream two groups later so PE never stalls waiting on ACT.
            # Epilogues are delayed 3 groups past their chunk's last PV.
            def emit_body():
                step = [0]
                pvq = []        # deferred PV emitters (depth 2)
                delayed = []    # (due_step, fn) epilogues

                def tick():
                    step[0] += 1
                    for due, fn in [d for d in delayed if d[0] <= step[0]]:
                        delayed.remove((due, fn))
                        fn()
                    if len(pvq) >= 2:
                        pvq.pop(0)()

                for p in range(NP):
                    pair_prologue(p)
                    for s0, sw in _chunks(SP, CHUNK):
                        pv_ps = pv_psum.tile([D + 1, CHUNK], F32, tag="pvp")
                        for tg in range(NT // 2):
                            qk_ps = emit_qk(p, s0, sw, tg)
                            exp_sb = emit_exp(p, sw, qk_ps)
                            tick()
                            pvq.append(make_pv(p, sw, tg, exp_sb, pv_ps))
                        delayed.append((step[0] + 3, make_epilogue(p, s0, sw, pv_ps)))
                while pvq:
                    pvq.pop(0)()
                for _, fn in delayed:
                    fn()

            if repeat == 1:
                emit_body()
            else:
                with tc.For_i(0, repeat, 1):
                    emit_body()

    nc.compile()
    return nc


B, H = 2, 16
S, D = 2048, 64
N_CORES = 8
PAIRS_PER_CORE = (B * H) // N_CORES  # 4

_NC_CACHE = {}
last_results = None  # BassKernelResults of the most recent kernel() call


def _install_profile_hook():
    """Wire up the axon NTFF profiling hook if the image's antenv lacks it."""
    import types

    try:
        import antenv.axon_hooks  # noqa: F401

        return
    except ImportError:
        pass
    try:
        from trn_agent_boot.trn_boot import _ntff_profile_via_ctypes

        hook = _ntff_profile_via_ctypes("/opt/axon/libaxon_pjrt.so")
    except Exception:
        hook = None
    mod = types.ModuleType("antenv.axon_hooks")
    mod._hook = hook
    mod.get_axon_ntff_profile_hook = lambda: mod._hook
    mod.set_axon_ntff_profile_hook = lambda h: setattr(mod, "_hook", h)
    sys.modules["antenv.axon_hooks"] = mod
    import antenv

    antenv.axon_hooks = mod
    # artifact upload needs a bucket this container doesn't have
    import concourse.bass_utils as _bu

    _bu.upload_artifacts = lambda tmpdir: "local://" + tmpdir


def kernel(query, key, value, mask):
    """Full-input attention; shards over 8 NeuronCores internally."""
    global last_results
    query = np.ascontiguousarray(np.asarray(query, dtype=np.float32))
    key = np.ascontiguousarray(np.asarray(key, dtype=np.float32))
    value = np.ascontiguousarray(np.asarray(value, dtype=np.float32))
    mask = np.asarray(mask)

    idx = [np.nonzero(mask[b] != 0)[0] for b in range(B)]
    cnt = [len(ix) for ix in idx]
    SP = max(128, -(-max(cnt) // 128) * 128)
    # ensure at least one zero-padded row exists wherever masked rows need
    # a colmean(V) fill
    if any(c < S and c == SP for c in cnt):
        SP += 128

    nc = _NC_CACHE.get(SP)
    if nc is None:
        nc = _NC_CACHE[SP] = build_attention_nc(NP=PAIRS_PER_CORE, SP=SP)

    in_maps = []
    for c in range(N_CORES):
        qs = np.zeros((PAIRS_PER_CORE, SP, D), dtype=np.float32)
        import ml_dtypes

        ks = np.empty((PAIRS_PER_CORE, D, S), dtype=ml_dtypes.bfloat16)
        vs = np.empty((PAIRS_PER_CORE, S, D), dtype=ml_dtypes.bfloat16)
        for i in range(PAIRS_PER_CORE):
            pair = c * PAIRS_PER_CORE + i
            b, h = pair // H, pair % H
            qs[i, : cnt[b]] = query[b, h, idx[b]]
            ks[i] = key[b, h]
            vs[i] = value[b, h]
        in_maps.append({"q": qs, "kt": ks, "v": vs})

    trace = os.environ.get("KERNEL_PROFILE", "") == "1"
    if trace:
        _install_profile_hook()
        try:
            # the NTFF hook needs the axon PJRT client initialized by a real
            # device interaction before axon_start_nrt_profile works
            import jax

            jax.device_put(
                np.zeros((4,), np.float32), jax.devices()[0]
            ).block_until_ready()
        except Exception as e:
            print(f"profile warmup failed ({e}); disabling trace", file=sys.stderr)
            trace = False
    res = run_bass_kernel_spmd(nc, in_maps, core_ids=list(range(N_CORES)), trace=trace)
    last_results = res

    out = np.empty((B, H, S, D), dtype=np.float32)
    for c in range(N_CORES):
        oc = res.results[c]["o"]
        for i in range(PAIRS_PER_CORE):
            pair = c * PAIRS_PER_CORE + i
            b, h = pair // H, pair % H
            out[b, h, idx[b]] = oc[i, : cnt[b]]
            if cnt[b] < S:
                out[b, h, np.nonzero(mask[b] == 0)[0]] = oc[i, cnt[b]]
    return out



# revision 9
# speedup vs baseline: 1.5731x; 1.5731x over previous
"""TRN2 Bass kernel for nn_Attention_11252814315826.

out[b,h,s,:] = softmax(Q[b,h] @ K^T[b,h] / 8 + addr(mask)) @ V[b,h]
with the additive mask on the QUERY dim: for mask[b,s]==0 the reference's
-1e12 row offset makes softmax exactly uniform, so out = colmean(V[b,h]).

Strategy (v2): shard the 32 (b,h) pairs 4-per-core across 8 NeuronCores.
Host-side, compact query rows to the mask==1 subset, transpose to qT
[64, SP] bf16 (SP = max_cnt+1; one zero column supplies colmean(V) for
all masked rows), and duplicate qT/kT onto both PE row halves so QK
matmuls run row-tiled: two key-blocks concurrently on PE rows 0-63 /
64-127 (K=64 each) for ~2x QK throughput.

Scores for 3 key-blocks accumulate in one [128, 3*512] PSUM tile so a
single ACTIVATE(Exp) covers N=1536 (amortizes the ~310-cycle ScalarE
per-instruction overhead; exp is the roofline engine at 1 elem/cycle/
lane @ 1.2 GHz). PV accumulates [65, sw] (V plus a ones column for the
softmax denominator) in PSUM over all 16 key-blocks; epilogue is a DVE
copy PSUM->SBUF and a DMA of the raw [65, SP] (d-major) result. The
host divides by the denominator row and transposes/scatters back.
"""

import os
import sys

for _p in (
    "/root/.axon_site",
    "/root/.axon_site/_ro/trn_rl_repo",
    "/root/.axon_site/_ro/pypackages",
    "/opt/trn_rl_repo",
):
    if os.path.isdir(_p) and _p not in sys.path:
        sys.path.append(_p)

from concourse.bass_utils import run_bass_kernel_spmd

import numpy as np

import concourse.bacc as bacc
import concourse.tile as tile
import concourse.mybir as mybir

F32 = mybir.dt.float32
BF16 = mybir.dt.bfloat16

B, H = 2, 16
S, D = 2048, 64
N_CORES = 8
PAIRS_PER_CORE = (B * H) // N_CORES  # 4
CH = 512  # max chunk width (psum bank = 512 f32)


def _chunks(total, size):
    out, s0 = [], 0
    while s0 < total:
        w = min(size, total - s0)
        out.append((s0, w))
        s0 += w
    return out


def _groups(sw, nt):
    """Split nt key-blocks into ACT groups. Full chunks use 3-block groups
    (N=1536 exp per ACTIVATE, 3 psum banks); narrow chunks pack up to
    8 blocks while each matmul output stays inside one 2KB psum bank."""
    if sw == CH:
        sizes = []
        left = nt
        while left > 0:
            g = min(3, left)
            sizes.append(g)
            left -= g
    else:
        # keep every [128, sw] f32 matmul output within one bank
        per = max(1, min(nt, 2048 // (sw * 4)))
        sizes = []
        left = nt
        while left > 0:
            g = min(per, left)
            sizes.append(g)
            left -= g
    out, t0 = [], 0
    for g in sizes:
        out.append((t0, g))
        t0 += g
    return out


def build_attention_nc(NP=4, SP=1047, S_=2048, D_=64, row_tile=True, repeat=1):
    assert D_ == 64
    NT = S_ // 128

    nc = bacc.Bacc("TRN2", target_bir_lowering=False, debug=False)

    qt = nc.dram_tensor("qt", [NP, D_, SP], BF16, kind="ExternalInput")
    kt = nc.dram_tensor("kt", [NP, D_, S_], BF16, kind="ExternalInput")
    v = nc.dram_tensor("v", [NP, S_, D_], BF16, kind="ExternalInput")
    o = nc.dram_tensor("o", [NP, D_ + 1, SP], F32, kind="ExternalOutput")

    import ml_dtypes

    ones_dram = nc.inline_tensor(
        np.ones((128, NT, 1), dtype=ml_dtypes.bfloat16), name="onescol"
    )
    dma = nc.sync

    chunks = _chunks(SP, CH)

    with tile.TileContext(nc) as tc:
        with (
            tc.tile_pool(name="kt", bufs=2) as kt_pool,
            tc.tile_pool(name="v", bufs=2) as v_pool,
            tc.tile_pool(name="qt", bufs=2) as qt_pool,
            tc.tile_pool(name="exp", bufs=3) as exp_pool,
            tc.tile_pool(name="osb", bufs=2) as osb_pool,
            tc.tile_pool(name="qkps", bufs=2, space="PSUM") as qk_psum,
            tc.tile_pool(name="pvps", bufs=2, space="PSUM") as pv_psum,
        ):
            ctxs = {}

            def pair_prologue(p):
                # kT duplicated on both partition halves for row-tiling
                kt_sb = kt_pool.tile([128, S_], BF16, tag="kt")
                for h in range(2):
                    for c0 in range(0, S_, S_ // 2):
                        dma.dma_start(
                            kt_sb[h * 64 : h * 64 + 64, c0 : c0 + S_ // 2],
                            kt.ap()[p][:, c0 : c0 + S_ // 2],
                        )
                v_sb = v_pool.tile([128, NT, D_ + 1], BF16, tag="v")
                v_src = v.ap()[p].rearrange("(t p) d -> p t d", p=128)
                for t0 in range(0, NT, NT // 2):
                    dma.dma_start(
                        v_sb[:, t0 : t0 + NT // 2, 0:D_],
                        v_src[:, t0 : t0 + NT // 2, :],
                    )
                dma.dma_start(v_sb[:, :, D_ : D_ + 1], ones_dram.ap())
                # qT duplicated on both halves
                qt_sb = qt_pool.tile([128, SP], BF16, tag="qt")
                for h in range(2):
                    dma.dma_start(qt_sb[h * 64 : h * 64 + 64, :], qt.ap()[p])
                ctxs[p] = dict(kt=kt_sb, v=v_sb, qt=qt_sb)

            def emit_qk_group(p, s0, sw, t0g, glen):
                """QK matmuls for key-blocks t0g..t0g+glen-1, row-tiled by
                parity (half 0 / half 1 run concurrently on the PE)."""
                cx = ctxs[p]
                kt_sb, qt_sb = cx["kt"], cx["qt"]
                qk_t = qk_psum.tile([128, 3 * CH], F32, tag="qk")
                for j in range(glen):
                    t = t0g + j
                    h = (t % 2) if row_tile else 0
                    nc.tensor.matmul(
                        qk_t[:, j * sw : (j + 1) * sw],
                        kt_sb[h * 64 : h * 64 + 64, t * 128 : (t + 1) * 128],
                        qt_sb[h * 64 : h * 64 + 64, s0 : s0 + sw],
                        start=True,
                        stop=True,
                        tile_position=(h * 64, 0) if row_tile else None,
                    )
                return qk_t

            def emit_exp(p, sw, glen, qk_t):
                exp_t = exp_pool.tile([128, 3 * CH], BF16, tag="exp")
                nc.scalar.activation(
                    exp_t[:, 0 : glen * sw],
                    qk_t[:, 0 : glen * sw],
                    mybir.ActivationFunctionType.Exp,
                    scale=0.125,
                )
                return exp_t

            def make_pv(p, sw, t0g, glen, exp_t, pv_t):
                def emit():
                    v_sb = ctxs[p]["v"]
                    for j in range(glen):
                        t = t0g + j
                        nc.tensor.matmul(
                            pv_t[:, 0:sw],
                            v_sb[:, t, :],
                            exp_t[:, j * sw : (j + 1) * sw],
                            start=(t == 0),
                            stop=(t == NT - 1),
                            skip_group_check=True,
                        )

                return emit

            def make_epilogue(p, s0, sw, pv_t):
                def emit():
                    o_sb = osb_pool.tile([D_ + 1, CH], F32, tag="osb")
                    nc.vector.tensor_copy(o_sb[:, 0:sw], pv_t[:, 0:sw])
                    dma.dma_start(o.ap()[p][:, s0 : s0 + sw], o_sb[:, 0:sw])

                return emit

            # flat software-pipelined emission: PE stream is
            # [... QK(g), PV(g-2), QK(g+1), PV(g-1) ...] so the scalar
            # engine's exp(g) hides under PE work and never gates QK.
            def emit_body():
                gstep = [0]
                pvq = []
                epiq = []

                def tick():
                    gstep[0] += 1
                    while epiq and epiq[0][0] <= gstep[0]:
                        epiq.pop(0)[1]()
                    if len(pvq) >= 2:
                        pvq.pop(0)()

                for p in range(NP):
                    pair_prologue(p)
                    for s0, sw in chunks:
                        pv_t = pv_psum.tile([D_ + 1, CH], F32, tag="pv")
                        for t0g, glen in _groups(sw, NT):
                            qk_t = emit_qk_group(p, s0, sw, t0g, glen)
                            exp_t = emit_exp(p, sw, glen, qk_t)
                            tick()
                            pvq.append(make_pv(p, sw, t0g, glen, exp_t, pv_t))
                        epiq.append((gstep[0] + 3, make_epilogue(p, s0, sw, pv_t)))
                while pvq:
                    pvq.pop(0)()
                for _, fn in epiq:
                    fn()

            if repeat == 1:
                emit_body()
            else:
                with tc.For_i(0, repeat, 1):
                    emit_body()

    nc.compile()
    return nc


_NC_CACHE = {}
last_results = None


def _install_profile_hook():
    """Wire up the axon NTFF profiling hook if the image's antenv lacks it."""
    import types

    try:
        import antenv.axon_hooks  # noqa: F401

        return
    except ImportError:
        pass
    try:
        from trn_agent_boot.trn_boot import _ntff_profile_via_ctypes

        hook = _ntff_profile_via_ctypes("/opt/axon/libaxon_pjrt.so")
    except Exception:
        hook = None
    mod = types.ModuleType("antenv.axon_hooks")
    mod._hook = hook
    mod.get_axon_ntff_profile_hook = lambda: mod._hook
    mod.set_axon_ntff_profile_hook = lambda h: setattr(mod, "_hook", h)
    sys.modules["antenv.axon_hooks"] = mod
    import antenv

    antenv.axon_hooks = mod
    import concourse.bass_utils as _bu

    _bu.upload_artifacts = lambda tmpdir: "local://" + tmpdir


def kernel(query, key, value, mask):
    """Full-input attention; shards over 8 NeuronCores internally."""
    global last_results
    import ml_dtypes

    query = np.asarray(query)
    key = np.asarray(key)
    value = np.asarray(value)
    mask = np.asarray(mask)

    idx = [np.nonzero(mask[b] != 0)[0] for b in range(B)]
    cnt = [len(ix) for ix in idx]
    # one zero-padded qT column per batch supplies colmean(V) for masked rows;
    # keep SP even so bf16 DMA rows stay 4-byte aligned
    SP = max(cnt) + (1 if min(cnt) < S else 0)
    SP += SP % 2

    nc = _NC_CACHE.get(SP)
    if nc is None:
        nc = _NC_CACHE[SP] = build_attention_nc(
            NP=PAIRS_PER_CORE,
            SP=SP,
            row_tile=os.environ.get("KERNEL_ROW_TILE", "1") == "1",
        )

    in_maps = []
    for c in range(N_CORES):
        qs = np.zeros((PAIRS_PER_CORE, D, SP), dtype=ml_dtypes.bfloat16)
        ks = np.empty((PAIRS_PER_CORE, D, S), dtype=ml_dtypes.bfloat16)
        vs = np.empty((PAIRS_PER_CORE, S, D), dtype=ml_dtypes.bfloat16)
        for i in range(PAIRS_PER_CORE):
            pair = c * PAIRS_PER_CORE + i
            b, h = pair // H, pair % H
            qs[i, :, : cnt[b]] = query[b, h, idx[b]].T
            ks[i] = key[b, h]
            vs[i] = value[b, h]
        in_maps.append({"qt": qs, "kt": ks, "v": vs})

    trace = os.environ.get("KERNEL_PROFILE", "") == "1"
    if trace:
        _install_profile_hook()
        try:
            import jax

            jax.device_put(
                np.zeros((4,), np.float32), jax.devices()[0]
            ).block_until_ready()
        except Exception as e:
            print(f"profile warmup failed ({e}); disabling trace", file=sys.stderr)
            trace = False
    res = run_bass_kernel_spmd(nc, in_maps, core_ids=list(range(N_CORES)), trace=trace)
    last_results = res

    out = np.empty((B, H, S, D), dtype=np.float32)
    for c in range(N_CORES):
        oc = res.results[c]["o"]  # [NP, D+1, SP] f32 (raw PV + denominator row)
        for i in range(PAIRS_PER_CORE):
            pair = c * PAIRS_PER_CORE + i
            b, h = pair // H, pair % H
            on = oc[i, :D, :] / oc[i, D : D + 1, :]
            out[b, h, idx[b]] = on[:, : cnt[b]].T
            if cnt[b] < S:
                out[b, h, np.nonzero(mask[b] == 0)[0]] = on[:, cnt[b]]
    return out
